# revision 23
# baseline (speedup 1.0000x reference)
"""Trainium2 Bass kernel for nn_MemoryWriteHead (DNC-style memory write head).

Computes, per batch row:
  - tiny projections of h (write key/strength/erase/add/gates)
  - cosine content addressing over memory (B, N, M) + softmax
  - DNC allocation weighting: argsort usage, cumprod, scatter back
  - write_weights = wg * (ag * alloc + (1 - ag) * content)

Strategy: pure data parallel over 8 NeuronCores (32 batch rows each).
The argsort is an all-ascending bitonic sorting network on the DVE
(values sorted exactly on the 2^-23 uniform grid), with an index payload
moved by copy_predicated; per-stage swap masks are archived and replayed
in reverse to scatter allocation weights back to original positions.
Ties are fixed exactly with odd-even transposition passes on the index
payload. The cosine-sim dot products run on the TensorEngine against a
host-pre-transposed memory layout (m on partitions), so no on-chip
transposes of the big tensor are needed.
"""

import os
import sys

sys.path.insert(0, "/opt/trn_rl_repo")

import numpy as np

import concourse.bass as bass
import concourse.bacc as bacc
import concourse.mybir as mybir
import concourse.tile as tile
from concourse.masks import make_identity

B, N, M, H = 256, 2048, 128, 512
NCORES = 8
RB = B // NCORES          # 32 rows per core
S_CHUNKS = 4              # sort layout: 4 chunks of 512 per row
FCH = N // S_CHUNKS       # 512 free elems per chunk
TIE_PASSES = 1
V_SCALE = float(2.0 ** 23)
F32 = mybir.dt.float32
BF16 = mybir.dt.bfloat16
U8 = mybir.dt.uint8
Alu = mybir.AluOpType
Act = mybir.ActivationFunctionType


# ----------------------------------------------------------------------------
# sorting network description (python-side metadata)
# ----------------------------------------------------------------------------

def _sort_specs():
    """Instruction specs for the all-ascending bitonic network on 2048 elems.

    Element e = s*512 + f lives at partition s*32+r (r = row), free col f.
    Each spec: (kind, args) where kind selects the AP construction:
      ('xor_f', d)            pairs (f, f+d) within chunk, all partitions
      ('xor_p', pa, pb, cnt)  partition-offset pairs, full free dim
      ('refl_f', blk)         reflected pairs within free-dim blocks
      ('refl_p', pa, pb, cnt) partition pairs with reversed free dim
    Returns list of stages; each stage is a list of specs.
    """
    stages = []
    for k in range(1, 12):
        blk = 1 << k
        if blk == 2:
            stages.append([("xor_f", 1)])
        elif blk <= FCH:
            stages.append([("refl_f", blk)])
        elif blk == 2 * FCH:   # blocks (s0,s1), (s2,s3)
            stages.append([("refl_p", 0, 32, 32), ("refl_p", 64, 96, 32)])
        else:                  # blk == 4*FCH: pairs s0<->s3, s1<->s2
            stages.append([("refl_p", 0, 96, 32), ("refl_p", 32, 64, 32)])
        d = blk // 4
        while d >= 1:
            if d <= FCH // 2:
                stages.append([("xor_f", d)])
            else:  # d == 512: s-distance 1
                stages.append([("xor_p", 0, 32, 32), ("xor_p", 64, 96, 32)])
            d //= 2
    return stages


def _tie_specs():
    """Odd-even transposition passes over sorted slots (index payload only)."""
    stages = []
    for _ in range(TIE_PASSES):
        stages.append([("tie_even",)])
        stages.append([("tie_odd",), ("tie_bound",)])
    return stages


def _spec_aps(spec, t):
    """Given a spec and a [128, FCH] tile AP, return an op descriptor dict.

    Keys:
      A: AP for the low side (on the real tile)
      B: AP for the high side (real tile) -- only when stage is None
      mshape: mask tile shape
      mview: fn(mask_tile) -> AP matching A's dims
      stage: None, or dict(pa, cnt, src_off, w, rev) -- the B side lives at a
        different base partition; it must be DMA-staged into a scratch tile
        at base partition pa before ops, and DMA'd back after.
        src(t) gives the B-side slice on the real tile.
    """
    kind = spec[0]
    if kind == "xor_f":
        d = spec[1]
        v = t[:].rearrange("p (x two d) -> p x two d", two=2, d=d)
        x = FCH // (2 * d)
        mv = (lambda mk, d=d, x=x: mk[:].rearrange("p (d x) -> p x d", x=x))
        return dict(A=v[:, :, 0, :], B=v[:, :, 1, :], mshape=(128, FCH // 2),
                    mview=mv, stage=None)
    if kind == "refl_f":
        blk = spec[1]
        v = t[:].rearrange("p (x blk) -> p x blk", blk=blk)
        a = v[:, :, 0:blk // 2]
        b = v[:, :, blk - 1:blk // 2 - 1:-1]
        x = FCH // blk
        mv = (lambda mk, x=x: mk[:].rearrange("p (h x) -> p x h", x=x))
        return dict(A=a, B=b, mshape=(128, FCH // 2), mview=mv, stage=None)
    if kind == "tie_even":
        v = t[:].rearrange("p (x two) -> p x two", two=2)
        return dict(A=v[:, :, 0], B=v[:, :, 1], mshape=(128, FCH // 2),
                    mview=(lambda mk: mk[:]), stage=None)
    if kind == "tie_odd":
        v = t[:, 1:FCH - 1].rearrange("p (x two) -> p x two", two=2)
        return dict(A=v[:, :, 0], B=v[:, :, 1], mshape=(128, (FCH - 2) // 2),
                    mview=(lambda mk: mk[:]), stage=None)
    if kind == "xor_p":
        _, pa, pb, cnt = spec
        return dict(A=t[pa:pa + cnt, :], mshape=(128, FCH),
                    mview=(lambda mk, pa=pa, cnt=cnt: mk[pa:pa + cnt, :]),
                    stage=dict(pa=pa, cnt=cnt, src=t[pb:pb + cnt, :], w=FCH, rev=False))
    if kind == "refl_p":
        _, pa, pb, cnt = spec
        return dict(A=t[pa:pa + cnt, :], mshape=(128, FCH),
                    mview=(lambda mk, pa=pa, cnt=cnt: mk[pa:pa + cnt, :]),
                    stage=dict(pa=pa, cnt=cnt, src=t[pb:pb + cnt, :], w=FCH, rev=True))
    if kind == "tie_bound":
        return dict(A=t[0:96, FCH - 1:FCH], mshape=(128, 1),
                    mview=(lambda mk: mk[0:96, :]),
                    stage=dict(pa=0, cnt=96, src=t[32:128, 0:1], w=1, rev=False))
    raise ValueError(spec)


# ----------------------------------------------------------------------------
# program construction
# ----------------------------------------------------------------------------

def _emit(nc, tc):
    g = {}  # named DRAM tensors

    def din(name, shape):
        g[name] = nc.dram_tensor(name, shape, F32, kind="ExternalInput")

    def dout(name, shape):
        g[name] = nc.dram_tensor(name, shape, F32, kind="ExternalOutput")

    din("h", [RB, H])
    din("memT", [RB, M, N])
    din("usage", [RB, N])
    for w, b in (("Wk", "bk"), ("We", "be"), ("Wa", "ba")):
        din(w, [M, H])
        din(b, [M])
    for w, b in (("Ws", "bs"), ("Wg", "bg"), ("Wag", "bag")):
        din(w, [1, H])
        din(b, [1])
    din("idx", [128, FCH])
    dout("ww", [RB, N])
    dout("erase", [RB, M])
    dout("addv", [RB, M])
    dout("alloc", [RB, N])

    sort_stages = _sort_specs()
    tie_stages = _tie_specs()

    from contextlib import ExitStack
    es = ExitStack()
    with es:
        const = es.enter_context(tc.tile_pool(name="const", bufs=1))
        sortp = es.enter_context(tc.tile_pool(name="sortp", bufs=1))
        maskp = es.enter_context(tc.tile_pool(name="maskp", bufs=1))
        scr = es.enter_context(tc.tile_pool(name="scr", bufs=6))
        memp = es.enter_context(tc.tile_pool(name="memp", bufs=4))
        sqp = es.enter_context(tc.tile_pool(name="sqp", bufs=2))
        rmp = es.enter_context(tc.tile_pool(name="rmp", bufs=1))

        # ------------------------------------------------------------------
        # constants
        # ------------------------------------------------------------------
        ident = const.tile([128, 128], F32, tag="ident")
        make_identity(nc, ident[:])
        ones_col = const.tile([128, 1], F32, tag="ones")
        nc.gpsimd.memset(ones_col[:], 1.0)

        # ------------------------------------------------------------------
        # projections (PE)
        # ------------------------------------------------------------------
        with tc.tile_pool(name="prjps", bufs=2, space="PSUM") as prjps, \
             tc.tile_pool(name="prjacc", bufs=1, space="PSUM") as prjacc, \
             tc.tile_pool(name="prjsb", bufs=1) as prjsb:
            h_sb = prjsb.tile([RB, H], F32, tag="h")
            nc.sync.dma_start(h_sb[:], g["h"].ap())
            # hT: 4 transposes of [32,128] -> [128,32]
            hT = prjsb.tile([128, 4 * RB], F32, tag="hT")
            for kk in range(4):
                ps = prjps.tile([128, RB], F32, tag="tp", padded_shape=[128, 128])
                nc.tensor.transpose(ps[:], h_sb[:, kk * 128:(kk + 1) * 128], ident[0:RB, 0:RB])
                nc.scalar.copy(hT[:, kk * RB:(kk + 1) * RB], ps[:])

            # W transposes for Wk/We/Wa
            wT = {}
            for w in ("Wk", "We", "Wa"):
                w_sb = prjsb.tile([M, H], F32, tag="wld")
                nc.sync.dma_start(w_sb[:], g[w].ap())
                wt = prjsb.tile([128, 4 * 128], F32, tag=f"{w}T")
                for kk in range(4):
                    ps = prjps.tile([128, 128], F32, tag="tp")
                    nc.tensor.transpose(ps[:], w_sb[:, kk * 128:(kk + 1) * 128], ident[:])
                    nc.scalar.copy(wt[:, kk * 128:(kk + 1) * 128], ps[:])
                wT[w] = wt

            # scalar-projection weight columns: [128, 3] per k-tile
            svec = prjsb.tile([128, 4 * 3], F32, tag="svec")
            for j, w in enumerate(("Ws", "Wg", "Wag")):
                for kk in range(4):
                    nc.sync.dma_start(
                        svec[:, kk * 3 + j:kk * 3 + j + 1],
                        g[w].ap()[:, kk * 128:(kk + 1) * 128].rearrange("o k -> k o"),
                    )

            # biases
            bk128 = prjsb.tile([128, 1], F32, tag="bk128")
            nc.sync.dma_start(bk128[:], g["bk"].ap().rearrange("(m o) -> m o", o=1))
            brow = {}
            for b in ("bk", "be", "ba"):
                t = prjsb.tile([RB, M], F32, tag=f"{b}row")
                nc.sync.dma_start(t[:], g[b].ap().rearrange("(o m) -> o m", o=1).broadcast_to([RB, M]))
                brow[b] = t
            bsc = {}
            for b in ("bs", "bg", "bag"):
                t = prjsb.tile([RB, 1], F32, tag=f"{b}c")
                nc.sync.dma_start(t[:], g[b].ap().rearrange("(o p) -> o p", p=1).broadcast_to([RB, 1]))
                bsc[b] = t

            # matmuls
            ps_wkT = prjacc.tile([128, RB], F32, tag="wkT")
            ps_rm = {w: prjacc.tile([RB, 128], F32, tag=f"rm{w}", name=f"ps_rm_{w}") for w in ("Wk", "We", "Wa")}
            ps_sv = prjacc.tile([RB, 3], F32, tag="sv")
            for kk in range(4):
                hT_k = hT[:, kk * RB:(kk + 1) * RB]
                nc.tensor.matmul(ps_wkT[:], wT["Wk"][:, kk * 128:(kk + 1) * 128], hT_k,
                                 start=(kk == 0), stop=(kk == 3))
                for w in ("Wk", "We", "Wa"):
                    nc.tensor.matmul(ps_rm[w][:], hT_k, wT[w][:, kk * 128:(kk + 1) * 128],
                                     start=(kk == 0), stop=(kk == 3))
                nc.tensor.matmul(ps_sv[:], hT_k, svec[:, kk * 3:(kk + 1) * 3],
                                 start=(kk == 0), stop=(kk == 3))

            # epilogues of projections
            wkT_sb = const.tile([128, RB], F32, tag="wkT_sb")   # keep for sim phase
            nc.vector.tensor_scalar(wkT_sb[:], ps_wkT[:], bk128[:], None, Alu.add)

            wk_rm = rmp.tile([RB, 128], F32, tag="wk_rm")
            nc.vector.tensor_tensor(wk_rm[:], ps_rm["Wk"][:], brow["bk"][:], Alu.add)
            kn_scr = rmp.tile([RB, 128], F32, tag="kn_scr")
            k_nsq = rmp.tile([RB, 1], F32, tag="k_nsq")
            nc.scalar.activation(kn_scr[:], wk_rm[:], Act.Square, accum_out=k_nsq[:])
            k_norm = rmp.tile([RB, 1], F32, tag="k_norm")
            nc.scalar.activation(k_norm[:], k_nsq[:], Act.Sqrt)

            er_t = rmp.tile([RB, 128], F32, tag="er_t")
            nc.vector.tensor_tensor(er_t[:], ps_rm["We"][:], brow["be"][:], Alu.add)
            er_o = rmp.tile([RB, 128], F32, tag="er_o")
            nc.scalar.activation(er_o[:], er_t[:], Act.Sigmoid)
            nc.sync.dma_start(g["erase"].ap(), er_o[:])

            ad_o = rmp.tile([RB, 128], F32, tag="ad_o")
            nc.vector.tensor_tensor(ad_o[:], ps_rm["Wa"][:], brow["ba"][:], Alu.add)
            nc.sync.dma_start(g["addv"].ap(), ad_o[:])

            strength = rmp.tile([RB, 1], F32, tag="strength")
            nc.scalar.activation(strength[:], ps_sv[:, 0:1], Act.Exp, bias=bsc["bs"][:])
            nc.vector.tensor_scalar_add(strength[:], strength[:], 1.0)
            nc.scalar.activation(strength[:], strength[:], Act.Ln)
            wgate = rmp.tile([RB, 1], F32, tag="wgate")
            nc.scalar.activation(wgate[:], ps_sv[:, 1:2], Act.Sigmoid, bias=bsc["bg"][:])
            agate = rmp.tile([RB, 1], F32, tag="agate")
            nc.scalar.activation(agate[:], ps_sv[:, 2:3], Act.Sigmoid, bias=bsc["bag"][:])
            om_ag = rmp.tile([RB, 1], F32, tag="om_ag")
            nc.vector.tensor_scalar(om_ag[:], agate[:], -1.0, 1.0, Alu.mult, Alu.add)

        # ------------------------------------------------------------------
        # sort phase (DVE + ACT)
        # ------------------------------------------------------------------
        V = sortp.tile([128, FCH], F32, tag="V")
        nc.sync.dma_start(V[:], g["usage"].ap().rearrange("r (s f) -> s r f", s=S_CHUNKS))
        nc.vector.tensor_scalar_mul(V[:], V[:], V_SCALE)
        I = sortp.tile([128, FCH], F32, tag="I")
        nc.sync.dma_start(I[:], g["idx"].ap())

        all_replay = []  # (spec, mask_tile)

        _scr_n = [0]

        def scratch(tag, shape=(128, FCH)):
            _scr_n[0] += 1
            return scr.tile(list(shape), F32, tag=tag, name=f"scr{_scr_n[0]}",
                            padded_shape=[128, FCH])

        def stage_in(sp, tile_ap):
            """DMA the B side into a base-aligned scratch; return (ops_B_ap, writeback)."""
            st = sp["stage"]
            if st is None:
                return sp["B"], None
            stg = scratch("stg")
            pa, cnt, w = st["pa"], st["cnt"], st["w"]
            dst = stg[pa:pa + cnt, 0:w]
            nc.sync.dma_start(dst, st["src"])
            if st["rev"]:
                ops_b = stg[pa:pa + cnt, w - 1::-1]
            else:
                ops_b = dst
            return ops_b, (dst, st["src"])

        def stage_out(wb):
            if wb is not None:
                dst, src = wb
                nc.sync.dma_start(src, dst)

        def emit_swap_stage(spec, is_tie):
            spV = _spec_aps(spec, V)
            spI = _spec_aps(spec, I)
            mshape, mview = spV["mshape"], spV["mview"]
            VA = spV["A"]
            IA = spI["A"]
            VB, wbV = stage_in(spV, V)
            IB, wbI = stage_in(spI, I)
            tag = "t" if is_tie else "m"
            mk = maskp.tile(list(mshape), U8, tag=f"{tag}{len(all_replay)}",
                            name=f"mk{len(all_replay)}")
            mkv = mview(mk)
            if is_tie:
                eq = scratch("se", mshape)
                eqv = mview(eq)
                nc.vector.tensor_tensor(eqv, VA, VB, Alu.is_equal)
                gt = scratch("sg", mshape)
                gtv = mview(gt)
                nc.vector.tensor_tensor(gtv, IA, IB, Alu.is_gt)
                nc.vector.tensor_tensor(mkv, eqv, gtv, Alu.logical_and)
            else:
                nc.vector.tensor_tensor(mkv, VA, VB, Alu.is_gt)
                tv = scratch("sv", mshape)
                tvv = mview(tv)
                nc.scalar.copy(tvv, VA)
                nc.vector.tensor_tensor(VA, VA, VB, Alu.min)
                nc.vector.tensor_tensor(VB, tvv, VB, Alu.max)
            ti = scratch("si", mshape)
            tiv = mview(ti)
            nc.scalar.copy(tiv, IA)
            nc.vector.copy_predicated(IA, mkv, IB)
            nc.vector.copy_predicated(IB, mkv, tiv)
            stage_out(wbV)
            stage_out(wbI)
            all_replay.append((spec, mk))

        for stage in sort_stages:
            for spec in stage:
                emit_swap_stage(spec, is_tie=False)
        for stage in tie_stages:
            for spec in stage:
                emit_swap_stage(spec, is_tie=True)

        # ------------------------------------------------------------------
        # cumprod + allocation weights in sorted space
        # ------------------------------------------------------------------
        s_val = sortp.tile([128, FCH], F32, tag="s_val")
        nc.vector.tensor_scalar_mul(s_val[:], V[:], 1.0 / V_SCALE)
        cp = sortp.tile([128, FCH], F32, tag="cp")
        nc.vector.tensor_tensor_scan(cp[:], s_val[:], s_val[:], 1.0, Alu.mult, Alu.bypass)
        # cross-chunk exclusive products, computed in a [32, 4] layout
        # (per-row chunk totals brought to one partition set via tiny DMAs)
        T32 = sortp.tile([32, 4], F32, tag="T32")
        for sc in range(S_CHUNKS):
            nc.sync.dma_start(T32[:, sc:sc + 1], cp[sc * 32:(sc + 1) * 32, FCH - 1:FCH])
        excT = sortp.tile([32, 4], F32, tag="excT")
        nc.gpsimd.memset(excT[:, 0:1], 1.0)
        nc.vector.tensor_copy(excT[:, 1:2], T32[:, 0:1])
        nc.vector.tensor_tensor(excT[:, 2:3], T32[:, 1:2], excT[:, 1:2], Alu.mult)
        nc.vector.tensor_tensor(excT[:, 3:4], T32[:, 2:3], excT[:, 2:3], Alu.mult)
        exc = sortp.tile([128, 1], F32, tag="exc")
        for sc in range(S_CHUNKS):
            nc.sync.dma_start(exc[sc * 32:(sc + 1) * 32, 0:1], excT[:, sc:sc + 1])
        # padded (exclusive within chunk)
        padded = sortp.tile([128, FCH], F32, tag="padded")
        nc.gpsimd.memset(padded[:, 0:1], 1.0)
        nc.vector.tensor_copy(padded[:, 1:FCH], cp[:, 0:FCH - 1])
        # alloc_sorted = (1 - s) * padded * exc
        AL = sortp.tile([128, FCH], F32, tag="AL")
        nc.vector.tensor_scalar(AL[:], padded[:], exc[:], None, Alu.mult)
        om_s = sortp.tile([128, FCH], F32, tag="om_s")
        nc.vector.tensor_scalar(om_s[:], s_val[:], -1.0, 1.0, Alu.mult, Alu.add)
        nc.vector.tensor_tensor(AL[:], om_s[:], AL[:], Alu.mult)

        # ------------------------------------------------------------------
        # replay (unsort alloc back to original positions)
        # ------------------------------------------------------------------
        _skip_replay = bool(int(os.environ.get("MWH_SKIP_REPLAY", "0")))
        for spec, mk in (() if _skip_replay else tuple(reversed(all_replay))):
            sp = _spec_aps(spec, AL)
            AA = sp["A"]
            mkv = sp["mview"](mk)
            AB, wbA = stage_in(sp, AL)
            ta = scratch("sa", sp["mshape"])
            tav = sp["mview"](ta)
            nc.vector.tensor_copy(tav, AA)
            nc.vector.copy_predicated(AA, mkv, AB)
            nc.vector.copy_predicated(AB, mkv, tav)
            stage_out(wbA)

        # to row-major
        alloc_rm = rmp.tile([RB, N], F32, tag="alloc_rm")
        for s in range(S_CHUNKS):
            nc.sync.dma_start(alloc_rm[:, s * FCH:(s + 1) * FCH], AL[s * 32:(s + 1) * 32, :])
        nc.sync.dma_start(g["alloc"].ap(), alloc_rm[:])

        # ------------------------------------------------------------------
        # cosine-sim stream (PE + ACT): memory tiles as stationary operand
        # ------------------------------------------------------------------
        dotT = rmp.tile([128, 16 * RB], F32, tag="dotT")
        mnT = rmp.tile([128, 16 * RB], F32, tag="mnT")
        _skip_sim = bool(int(os.environ.get("MWH_SKIP_SIM", "0")))
        with tc.tile_pool(name="simps", bufs=2, space="PSUM") as simps:
            for b in (() if _skip_sim else range(RB)):
                mem_sb = memp.tile([M, N], F32, tag="mem")
                nc.sync.dma_start(mem_sb[:], g["memT"].ap()[b])
                sq_sb = sqp.tile([M, N], F32, tag="sq")
                nc.scalar.activation(sq_sb[:], mem_sb[:], Act.Square)
                ps_d = simps.tile([128, 16], F32, tag="psd")
                ps_m = simps.tile([128, 16], F32, tag="psm")
                for c in range(16):
                    nc.tensor.matmul(ps_d[:, c:c + 1],
                                     mem_sb[:, c * 128:(c + 1) * 128],
                                     wkT_sb[:, b:b + 1], start=True, stop=True)
                    nc.tensor.matmul(ps_m[:, c:c + 1],
                                     sq_sb[:, c * 128:(c + 1) * 128],
                                     ones_col[:], start=True, stop=True)
                nc.scalar.copy(dotT[:, b * 16:(b + 1) * 16], ps_d[:])
                nc.scalar.copy(mnT[:, b * 16:(b + 1) * 16], ps_m[:])
        # transpose [128, (b c)] -> row-major [32, (c p)]
        dot_rm = rmp.tile([RB, N], F32, tag="dot_rm")
        mn_rm = rmp.tile([RB, N], F32, tag="mn_rm")
        if _skip_sim:
            nc.gpsimd.memset(dotT[:], 0.5)
            nc.gpsimd.memset(mnT[:], 0.5)
        with tc.tile_pool(name="simps2", bufs=1, space="PSUM") as simps2:
            for (src, dst, tg) in ((dotT, dot_rm, "d"), (mnT, mn_rm, "m")):
                ps_t = simps2.tile([RB, N], F32, tag=f"pst{tg}")
                for c in range(16):
                    nc.tensor.transpose(
                        ps_t[:, c * 128:(c + 1) * 128],
                        src[:].rearrange("p (b c) -> p b c", c=16)[:, :, c],
                        ident[:])
                nc.scalar.copy(dst[:], ps_t[:])

        # ------------------------------------------------------------------
        # epilogue: softmax + combine (row-major [32, 2048], tiles reused)
        # ------------------------------------------------------------------
        rm_s = rmp.tile([RB, N], F32, tag="rm_s")
        # mn_rm -> m_norm (in place), then denom into rm_s, recip into mn_rm
        nc.scalar.activation(mn_rm[:], mn_rm[:], Act.Sqrt)
        nc.vector.tensor_scalar(rm_s[:], mn_rm[:], k_norm[:], 1e-8, Alu.mult, Alu.add)
        nc.vector.reciprocal(mn_rm[:], rm_s[:])
        # z = dot * recip * strength (in place in dot_rm)
        nc.vector.scalar_tensor_tensor(dot_rm[:], dot_rm[:], strength[:], mn_rm[:],
                                       Alu.mult, Alu.mult)
        rowmax = rmp.tile([RB, 1], F32, tag="rowmax")
        nc.vector.tensor_reduce(out=rowmax[:], in_=dot_rm[:], axis=mybir.AxisListType.X, op=Alu.max)
        negmax = rmp.tile([RB, 1], F32, tag="negmax")
        nc.vector.tensor_scalar_mul(negmax[:], rowmax[:], -1.0)
        rowsum = rmp.tile([RB, 1], F32, tag="rowsum")
        nc.scalar.activation(rm_s[:], dot_rm[:], Act.Exp, bias=negmax[:], accum_out=rowsum[:])
        rs_rec = rmp.tile([RB, 1], F32, tag="rs_rec")
        nc.vector.reciprocal(rs_rec[:], rowsum[:])
        nc.vector.tensor_scalar(rs_rec[:], rs_rec[:], om_ag[:], None, Alu.mult)
        # content*(1-ag) into rm_s; alloc*ag into mn_rm; combine; *wg
        nc.vector.tensor_scalar(rm_s[:], rm_s[:], rs_rec[:], None, Alu.mult)
        nc.vector.scalar_tensor_tensor(rm_s[:], alloc_rm[:], agate[:], rm_s[:],
                                       Alu.mult, Alu.add)
        nc.vector.tensor_scalar(rm_s[:], rm_s[:], wgate[:], None, Alu.mult)
        nc.sync.dma_start(g["ww"].ap(), rm_s[:])


_PROGRAM = None


def _get_program():
    global _PROGRAM
    if _PROGRAM is None:
        nc = bacc.Bacc(None, target_bir_lowering=False, debug=False, num_devices=NCORES)
        with tile.TileContext(nc) as tc:
            _emit(nc, tc)
        nc.compile()
        _PROGRAM = nc
    return _PROGRAM


def _idx_const():
    p = np.arange(128)[:, None]
    f = np.arange(FCH)[None, :]
    return ((p >> 5) * FCH + f).astype(np.float32)


def _shard_inputs(inputs):
    idx = _idx_const()
    memT = np.ascontiguousarray(np.transpose(np.asarray(inputs["memory"]), (0, 2, 1)))
    in_maps = []
    for c in range(NCORES):
        sl = slice(c * RB, (c + 1) * RB)
        m = {
            "h": np.ascontiguousarray(np.asarray(inputs["h"])[sl]),
            "memT": np.ascontiguousarray(memT[sl]),
            "usage": np.ascontiguousarray(np.asarray(inputs["prev_usage"])[sl]),
            "idx": idx,
        }
        for k in ("Wk", "bk", "We", "be", "Wa", "ba", "Ws", "bs", "Wg", "bg", "Wag", "bag"):
            m[k] = np.ascontiguousarray(np.asarray(inputs[k], dtype=np.float32))
        in_maps.append(m)
    return in_maps


def kernel(**inputs):
    from concourse.bass_utils import run_bass_kernel_spmd

    nc = _get_program()
    in_maps = _shard_inputs(inputs)
    trace = bool(int(os.environ.get("MWH_TRACE", "0")))
    res = run_bass_kernel_spmd(nc, in_maps, list(range(NCORES)), trace=trace)
    if trace and res.exec_time_ns is not None:
        kernel.last_exec_time_ns = res.exec_time_ns
    ww = np.concatenate([res.results[c]["ww"] for c in range(NCORES)], axis=0)
    erase = np.concatenate([res.results[c]["erase"] for c in range(NCORES)], axis=0)
    addv = np.concatenate([res.results[c]["addv"] for c in range(NCORES)], axis=0)
    alloc = np.concatenate([res.results[c]["alloc"] for c in range(NCORES)], axis=0)
    return ww, erase, addv, alloc


# revision 25
# speedup vs baseline: 1.0351x; 1.0351x over previous
"""Trainium2 Bass kernel for nn_MemoryWriteHead (DNC-style memory write head).

Computes, per batch row:
  - tiny projections of h (write key/strength/erase/add/gates)
  - cosine content addressing over memory (B, N, M) + softmax
  - DNC allocation weighting: argsort usage, cumprod, scatter back
  - write_weights = wg * (ag * alloc + (1 - ag) * content)

Strategy: pure data parallel over 8 NeuronCores (32 batch rows each).
The argsort is an all-ascending bitonic sorting network on the DVE
(values sorted exactly on the 2^-23 uniform grid), with an index payload
moved by copy_predicated; per-stage swap masks are archived and replayed
in reverse to scatter allocation weights back to original positions.
Ties are fixed exactly with odd-even transposition passes on the index
payload. The cosine-sim dot products run on the TensorEngine against a
host-pre-transposed memory layout (m on partitions), so no on-chip
transposes of the big tensor are needed.
"""

import os
import sys

sys.path.insert(0, "/opt/trn_rl_repo")

import numpy as np

import concourse.bass as bass
import concourse.bacc as bacc
import concourse.mybir as mybir
import concourse.tile as tile
from concourse.masks import make_identity

B, N, M, H = 256, 2048, 128, 512
NCORES = 8
RB = B // NCORES          # 32 rows per core
S_CHUNKS = 4              # sort layout: 4 chunks of 512 per row
FCH = N // S_CHUNKS       # 512 free elems per chunk
TIE_PASSES = 1
V_SCALE = float(2.0 ** 23)
F32 = mybir.dt.float32
BF16 = mybir.dt.bfloat16
U8 = mybir.dt.uint8
Alu = mybir.AluOpType
Act = mybir.ActivationFunctionType


# ----------------------------------------------------------------------------
# sorting network description (python-side metadata)
# ----------------------------------------------------------------------------

def _sort_specs():
    """Instruction specs for the all-ascending bitonic network on 2048 elems.

    Element e = s*512 + f lives at partition s*32+r (r = row), free col f.
    Each spec: (kind, args) where kind selects the AP construction:
      ('xor_f', d)            pairs (f, f+d) within chunk, all partitions
      ('xor_p', pa, pb, cnt)  partition-offset pairs, full free dim
      ('refl_f', blk)         reflected pairs within free-dim blocks
      ('refl_p', pa, pb, cnt) partition pairs with reversed free dim
    Returns list of stages; each stage is a list of specs.
    """
    stages = []
    for k in range(1, 12):
        blk = 1 << k
        if blk == 2:
            stages.append([("xor_f", 1)])
        elif blk <= FCH:
            stages.append([("refl_f", blk)])
        elif blk == 2 * FCH:   # blocks (s0,s1), (s2,s3)
            stages.append([("refl_p", 0, 32, 32), ("refl_p", 64, 96, 32)])
        else:                  # blk == 4*FCH: pairs s0<->s3, s1<->s2
            stages.append([("refl_p", 0, 96, 32), ("refl_p", 32, 64, 32)])
        d = blk // 4
        while d >= 1:
            if d <= FCH // 2:
                stages.append([("xor_f", d)])
            else:  # d == 512: s-distance 1
                stages.append([("xor_p", 0, 32, 32), ("xor_p", 64, 96, 32)])
            d //= 2
    return stages


def _tie_specs():
    """Odd-even transposition passes over sorted slots (index payload only)."""
    stages = []
    for _ in range(TIE_PASSES):
        stages.append([("tie_even",)])
        stages.append([("tie_odd",), ("tie_bound",)])
    return stages


def _spec_aps(spec, t):
    """Given a spec and a [128, FCH] tile AP, return an op descriptor dict.

    Keys:
      A: AP for the low side (on the real tile)
      B: AP for the high side (real tile) -- only when stage is None
      mshape: mask tile shape
      mview: fn(mask_tile) -> AP matching A's dims
      stage: None, or dict(pa, cnt, src_off, w, rev) -- the B side lives at a
        different base partition; it must be DMA-staged into a scratch tile
        at base partition pa before ops, and DMA'd back after.
        src(t) gives the B-side slice on the real tile.
    """
    kind = spec[0]
    if kind == "xor_f":
        d = spec[1]
        v = t[:].rearrange("p (x two d) -> p x two d", two=2, d=d)
        x = FCH // (2 * d)
        mv = (lambda mk, d=d, x=x: mk[:].rearrange("p (d x) -> p x d", x=x))
        fz = None if d >= FCH // 2 else dict(
            ov=(lambda t, d=d: t[:].rearrange("p (x two d) -> p two x d", two=2, d=d)),
            pv=(lambda t, d=d: t[:].rearrange("p (x two d) -> p two x d",
                                              two=2, d=d)[:, ::-1, :, :]),
            mv0=(lambda mk, d=d, x=x: mk[:].rearrange("p (d x) -> p x d", x=x)
                 .unsqueeze(1).broadcast_to([128, 2, x, d])),
        )
        return dict(A=v[:, :, 0, :], B=v[:, :, 1, :], mshape=(128, FCH // 2),
                    mview=mv, stage=None, fused=fz)
    if kind == "refl_f":
        blk = spec[1]
        v = t[:].rearrange("p (x blk) -> p x blk", blk=blk)
        a = v[:, :, 0:blk // 2]
        b = v[:, :, blk - 1:blk // 2 - 1:-1]
        x = FCH // blk
        mv = (lambda mk, x=x: mk[:].rearrange("p (h x) -> p x h", x=x))
        return dict(A=a, B=b, mshape=(128, FCH // 2), mview=mv, stage=None)
    if kind == "tie_even":
        v = t[:].rearrange("p (x two) -> p x two", two=2)
        fz = dict(
            ov=(lambda t: t[:].rearrange("p (x two) -> p two x", two=2)),
            pv=(lambda t: t[:].rearrange("p (x two) -> p two x", two=2)[:, ::-1, :]),
            mv0=(lambda mk: mk[:].unsqueeze(1).broadcast_to([128, 2, FCH // 2])),
        )
        return dict(A=v[:, :, 0], B=v[:, :, 1], mshape=(128, FCH // 2),
                    mview=(lambda mk: mk[:]), stage=None, fused=fz)
    if kind == "tie_odd":
        v = t[:, 1:FCH - 1].rearrange("p (x two) -> p x two", two=2)
        x2 = (FCH - 2) // 2
        fz = dict(
            ov=(lambda t: t[:, 1:FCH - 1].rearrange("p (x two) -> p two x", two=2)),
            pv=(lambda t: t[:, 1:FCH - 1].rearrange("p (x two) -> p two x",
                                                    two=2)[:, ::-1, :]),
            mv0=(lambda mk, x2=x2: mk[:].unsqueeze(1).broadcast_to([128, 2, x2])),
        )
        return dict(A=v[:, :, 0], B=v[:, :, 1], mshape=(128, (FCH - 2) // 2),
                    mview=(lambda mk: mk[:]), stage=None, fused=fz)
    if kind == "xor_p":
        _, pa, pb, cnt = spec
        return dict(A=t[pa:pa + cnt, :], mshape=(128, FCH),
                    mview=(lambda mk, pa=pa, cnt=cnt: mk[pa:pa + cnt, :]),
                    stage=dict(pa=pa, cnt=cnt, src=t[pb:pb + cnt, :], w=FCH, rev=False))
    if kind == "refl_p":
        _, pa, pb, cnt = spec
        return dict(A=t[pa:pa + cnt, :], mshape=(128, FCH),
                    mview=(lambda mk, pa=pa, cnt=cnt: mk[pa:pa + cnt, :]),
                    stage=dict(pa=pa, cnt=cnt, src=t[pb:pb + cnt, :], w=FCH, rev=True))
    if kind == "tie_bound":
        return dict(A=t[0:96, FCH - 1:FCH], mshape=(128, 1),
                    mview=(lambda mk: mk[0:96, :]),
                    stage=dict(pa=0, cnt=96, src=t[32:128, 0:1], w=1, rev=False))
    raise ValueError(spec)


# ----------------------------------------------------------------------------
# program construction
# ----------------------------------------------------------------------------

def _emit(nc, tc):
    g = {}  # named DRAM tensors

    def din(name, shape):
        g[name] = nc.dram_tensor(name, shape, F32, kind="ExternalInput")

    def dout(name, shape):
        g[name] = nc.dram_tensor(name, shape, F32, kind="ExternalOutput")

    din("h", [RB, H])
    din("memT", [RB, M, N])
    din("usage", [RB, N])
    for w, b in (("Wk", "bk"), ("We", "be"), ("Wa", "ba")):
        din(w, [M, H])
        din(b, [M])
    for w, b in (("Ws", "bs"), ("Wg", "bg"), ("Wag", "bag")):
        din(w, [1, H])
        din(b, [1])
    din("idx", [128, FCH])
    dout("ww", [RB, N])
    dout("erase", [RB, M])
    dout("addv", [RB, M])
    dout("alloc", [RB, N])

    sort_stages = _sort_specs()
    tie_stages = _tie_specs()

    from contextlib import ExitStack
    es = ExitStack()
    with es:
        const = es.enter_context(tc.tile_pool(name="const", bufs=1))
        sortp = es.enter_context(tc.tile_pool(name="sortp", bufs=1))
        maskp = es.enter_context(tc.tile_pool(name="maskp", bufs=1))
        scr = es.enter_context(tc.tile_pool(name="scr", bufs=6))
        memp = es.enter_context(tc.tile_pool(name="memp", bufs=4))
        sqp = es.enter_context(tc.tile_pool(name="sqp", bufs=2))
        rmp = es.enter_context(tc.tile_pool(name="rmp", bufs=1))

        # ------------------------------------------------------------------
        # constants
        # ------------------------------------------------------------------
        ident = const.tile([128, 128], F32, tag="ident")
        make_identity(nc, ident[:])
        ones_col = const.tile([128, 1], F32, tag="ones")
        nc.gpsimd.memset(ones_col[:], 1.0)

        # ------------------------------------------------------------------
        # projections (PE)
        # ------------------------------------------------------------------
        with tc.tile_pool(name="prjps", bufs=2, space="PSUM") as prjps, \
             tc.tile_pool(name="prjacc", bufs=1, space="PSUM") as prjacc, \
             tc.tile_pool(name="prjsb", bufs=1) as prjsb:
            h_sb = prjsb.tile([RB, H], F32, tag="h")
            nc.sync.dma_start(h_sb[:], g["h"].ap())
            # hT: 4 transposes of [32,128] -> [128,32]
            hT = prjsb.tile([128, 4 * RB], F32, tag="hT")
            for kk in range(4):
                ps = prjps.tile([128, RB], F32, tag="tp", padded_shape=[128, 128])
                nc.tensor.transpose(ps[:], h_sb[:, kk * 128:(kk + 1) * 128], ident[0:RB, 0:RB])
                nc.scalar.copy(hT[:, kk * RB:(kk + 1) * RB], ps[:])

            # W transposes for Wk/We/Wa
            wT = {}
            for w in ("Wk", "We", "Wa"):
                w_sb = prjsb.tile([M, H], F32, tag="wld")
                nc.sync.dma_start(w_sb[:], g[w].ap())
                wt = prjsb.tile([128, 4 * 128], F32, tag=f"{w}T")
                for kk in range(4):
                    ps = prjps.tile([128, 128], F32, tag="tp")
                    nc.tensor.transpose(ps[:], w_sb[:, kk * 128:(kk + 1) * 128], ident[:])
                    nc.scalar.copy(wt[:, kk * 128:(kk + 1) * 128], ps[:])
                wT[w] = wt

            # scalar-projection weight columns: [128, 3] per k-tile
            svec = prjsb.tile([128, 4 * 3], F32, tag="svec")
            for j, w in enumerate(("Ws", "Wg", "Wag")):
                for kk in range(4):
                    nc.sync.dma_start(
                        svec[:, kk * 3 + j:kk * 3 + j + 1],
                        g[w].ap()[:, kk * 128:(kk + 1) * 128].rearrange("o k -> k o"),
                    )

            # biases
            bk128 = prjsb.tile([128, 1], F32, tag="bk128")
            nc.sync.dma_start(bk128[:], g["bk"].ap().rearrange("(m o) -> m o", o=1))
            brow = {}
            for b in ("bk", "be", "ba"):
                t = prjsb.tile([RB, M], F32, tag=f"{b}row")
                nc.sync.dma_start(t[:], g[b].ap().rearrange("(o m) -> o m", o=1).broadcast_to([RB, M]))
                brow[b] = t
            bsc = {}
            for b in ("bs", "bg", "bag"):
                t = prjsb.tile([RB, 1], F32, tag=f"{b}c")
                nc.sync.dma_start(t[:], g[b].ap().rearrange("(o p) -> o p", p=1).broadcast_to([RB, 1]))
                bsc[b] = t

            # matmuls
            ps_wkT = prjacc.tile([128, RB], F32, tag="wkT")
            ps_rm = {w: prjacc.tile([RB, 128], F32, tag=f"rm{w}", name=f"ps_rm_{w}") for w in ("Wk", "We", "Wa")}
            ps_sv = prjacc.tile([RB, 3], F32, tag="sv")
            for kk in range(4):
                hT_k = hT[:, kk * RB:(kk + 1) * RB]
                nc.tensor.matmul(ps_wkT[:], wT["Wk"][:, kk * 128:(kk + 1) * 128], hT_k,
                                 start=(kk == 0), stop=(kk == 3))
                for w in ("Wk", "We", "Wa"):
                    nc.tensor.matmul(ps_rm[w][:], hT_k, wT[w][:, kk * 128:(kk + 1) * 128],
                                     start=(kk == 0), stop=(kk == 3))
                nc.tensor.matmul(ps_sv[:], hT_k, svec[:, kk * 3:(kk + 1) * 3],
                                 start=(kk == 0), stop=(kk == 3))

            # epilogues of projections
            wkT_sb = const.tile([128, RB], F32, tag="wkT_sb")   # keep for sim phase
            nc.vector.tensor_scalar(wkT_sb[:], ps_wkT[:], bk128[:], None, Alu.add)

            wk_rm = rmp.tile([RB, 128], F32, tag="wk_rm")
            nc.vector.tensor_tensor(wk_rm[:], ps_rm["Wk"][:], brow["bk"][:], Alu.add)
            kn_scr = rmp.tile([RB, 128], F32, tag="kn_scr")
            k_nsq = rmp.tile([RB, 1], F32, tag="k_nsq")
            nc.scalar.activation(kn_scr[:], wk_rm[:], Act.Square, accum_out=k_nsq[:])
            k_norm = rmp.tile([RB, 1], F32, tag="k_norm")
            nc.scalar.activation(k_norm[:], k_nsq[:], Act.Sqrt)

            er_t = rmp.tile([RB, 128], F32, tag="er_t")
            nc.vector.tensor_tensor(er_t[:], ps_rm["We"][:], brow["be"][:], Alu.add)
            er_o = rmp.tile([RB, 128], F32, tag="er_o")
            nc.scalar.activation(er_o[:], er_t[:], Act.Sigmoid)
            nc.sync.dma_start(g["erase"].ap(), er_o[:])

            ad_o = rmp.tile([RB, 128], F32, tag="ad_o")
            nc.vector.tensor_tensor(ad_o[:], ps_rm["Wa"][:], brow["ba"][:], Alu.add)
            nc.sync.dma_start(g["addv"].ap(), ad_o[:])

            strength = rmp.tile([RB, 1], F32, tag="strength")
            nc.scalar.activation(strength[:], ps_sv[:, 0:1], Act.Exp, bias=bsc["bs"][:])
            nc.vector.tensor_scalar_add(strength[:], strength[:], 1.0)
            nc.scalar.activation(strength[:], strength[:], Act.Ln)
            wgate = rmp.tile([RB, 1], F32, tag="wgate")
            nc.scalar.activation(wgate[:], ps_sv[:, 1:2], Act.Sigmoid, bias=bsc["bg"][:])
            agate = rmp.tile([RB, 1], F32, tag="agate")
            nc.scalar.activation(agate[:], ps_sv[:, 2:3], Act.Sigmoid, bias=bsc["bag"][:])
            om_ag = rmp.tile([RB, 1], F32, tag="om_ag")
            nc.vector.tensor_scalar(om_ag[:], agate[:], -1.0, 1.0, Alu.mult, Alu.add)

        # ------------------------------------------------------------------
        # sort phase (DVE + ACT)
        # ------------------------------------------------------------------
        V = sortp.tile([128, FCH], F32, tag="V")
        nc.sync.dma_start(V[:], g["usage"].ap().rearrange("r (s f) -> s r f", s=S_CHUNKS))
        nc.vector.tensor_scalar_mul(V[:], V[:], V_SCALE)
        I = sortp.tile([128, FCH], F32, tag="I")
        nc.sync.dma_start(I[:], g["idx"].ap())

        all_replay = []  # (spec, mask_tile)

        _scr_n = [0]

        def scratch(tag, shape=(128, FCH)):
            _scr_n[0] += 1
            return scr.tile(list(shape), F32, tag=tag, name=f"scr{_scr_n[0]}",
                            padded_shape=[128, FCH])

        def stage_in(sp, tile_ap):
            """DMA the B side into a base-aligned scratch; return (ops_B_ap, writeback)."""
            st = sp["stage"]
            if st is None:
                return sp["B"], None
            stg = scratch("stg")
            pa, cnt, w = st["pa"], st["cnt"], st["w"]
            dst = stg[pa:pa + cnt, 0:w]
            nc.sync.dma_start(dst, st["src"])
            if st["rev"]:
                ops_b = stg[pa:pa + cnt, w - 1::-1]
            else:
                ops_b = dst
            return ops_b, (dst, st["src"])

        def stage_out(wb):
            if wb is not None:
                dst, src = wb
                nc.sync.dma_start(src, dst)

        def emit_swap_stage(spec, is_tie):
            spV = _spec_aps(spec, V)
            spI = _spec_aps(spec, I)
            mshape, mview = spV["mshape"], spV["mview"]
            VA = spV["A"]
            IA = spI["A"]
            VB, wbV = stage_in(spV, V)
            IB, wbI = stage_in(spI, I)
            tag = "t" if is_tie else "m"
            mk = maskp.tile(list(mshape), U8, tag=f"{tag}{len(all_replay)}",
                            name=f"mk{len(all_replay)}")
            mkv = mview(mk)
            fz = spV.get("fused")
            if is_tie:
                eq = scratch("se", mshape)
                eqv = mview(eq)
                nc.vector.tensor_tensor(eqv, VA, VB, Alu.is_equal)
                gt = scratch("sg", mshape)
                gtv = mview(gt)
                nc.vector.tensor_tensor(gtv, IA, IB, Alu.is_gt)
                nc.vector.tensor_tensor(mkv, eqv, gtv, Alu.logical_and)
            else:
                nc.vector.tensor_tensor(mkv, VA, VB, Alu.is_gt)
                if fz is not None:
                    tv = scratch("sv")
                    nc.scalar.copy(tv[:], V[:])
                    nc.vector.copy_predicated(fz["ov"](V), fz["mv0"](mk), fz["pv"](tv))
                else:
                    tv = scratch("sv", mshape)
                    tvv = mview(tv)
                    nc.scalar.copy(tvv, VA)
                    nc.vector.tensor_tensor(VA, VA, VB, Alu.min)
                    nc.vector.tensor_tensor(VB, tvv, VB, Alu.max)
            if fz is not None:
                ti = scratch("si")
                nc.scalar.copy(ti[:], I[:])
                nc.vector.copy_predicated(fz["ov"](I), fz["mv0"](mk), fz["pv"](ti))
            else:
                ti = scratch("si", mshape)
                tiv = mview(ti)
                nc.scalar.copy(tiv, IA)
                nc.vector.copy_predicated(IA, mkv, IB)
                nc.vector.copy_predicated(IB, mkv, tiv)
            stage_out(wbV)
            stage_out(wbI)
            all_replay.append((spec, mk))

        for stage in sort_stages:
            for spec in stage:
                emit_swap_stage(spec, is_tie=False)
        for stage in tie_stages:
            for spec in stage:
                emit_swap_stage(spec, is_tie=True)

        # ------------------------------------------------------------------
        # cumprod + allocation weights in sorted space
        # ------------------------------------------------------------------
        s_val = sortp.tile([128, FCH], F32, tag="s_val")
        nc.vector.tensor_scalar_mul(s_val[:], V[:], 1.0 / V_SCALE)
        cp = sortp.tile([128, FCH], F32, tag="cp")
        nc.vector.tensor_tensor_scan(cp[:], s_val[:], s_val[:], 1.0, Alu.mult, Alu.bypass)
        # cross-chunk exclusive products, computed in a [32, 4] layout
        # (per-row chunk totals brought to one partition set via tiny DMAs)
        T32 = sortp.tile([32, 4], F32, tag="T32")
        for sc in range(S_CHUNKS):
            nc.sync.dma_start(T32[:, sc:sc + 1], cp[sc * 32:(sc + 1) * 32, FCH - 1:FCH])
        excT = sortp.tile([32, 4], F32, tag="excT")
        nc.gpsimd.memset(excT[:, 0:1], 1.0)
        nc.vector.tensor_copy(excT[:, 1:2], T32[:, 0:1])
        nc.vector.tensor_tensor(excT[:, 2:3], T32[:, 1:2], excT[:, 1:2], Alu.mult)
        nc.vector.tensor_tensor(excT[:, 3:4], T32[:, 2:3], excT[:, 2:3], Alu.mult)
        exc = sortp.tile([128, 1], F32, tag="exc")
        for sc in range(S_CHUNKS):
            nc.sync.dma_start(exc[sc * 32:(sc + 1) * 32, 0:1], excT[:, sc:sc + 1])
        # padded (exclusive within chunk)
        padded = sortp.tile([128, FCH], F32, tag="padded")
        nc.gpsimd.memset(padded[:, 0:1], 1.0)
        nc.vector.tensor_copy(padded[:, 1:FCH], cp[:, 0:FCH - 1])
        # alloc_sorted = (1 - s) * padded * exc
        AL = sortp.tile([128, FCH], F32, tag="AL")
        nc.vector.tensor_scalar(AL[:], padded[:], exc[:], None, Alu.mult)
        om_s = sortp.tile([128, FCH], F32, tag="om_s")
        nc.vector.tensor_scalar(om_s[:], s_val[:], -1.0, 1.0, Alu.mult, Alu.add)
        nc.vector.tensor_tensor(AL[:], om_s[:], AL[:], Alu.mult)

        # ------------------------------------------------------------------
        # replay (unsort alloc back to original positions)
        # ------------------------------------------------------------------
        _skip_replay = bool(int(os.environ.get("MWH_SKIP_REPLAY", "0")))
        for spec, mk in (() if _skip_replay else tuple(reversed(all_replay))):
            sp = _spec_aps(spec, AL)
            AA = sp["A"]
            mkv = sp["mview"](mk)
            AB, wbA = stage_in(sp, AL)
            ta = scratch("sa", sp["mshape"])
            tav = sp["mview"](ta)
            nc.vector.tensor_copy(tav, AA)
            nc.vector.copy_predicated(AA, mkv, AB)
            nc.vector.copy_predicated(AB, mkv, tav)
            stage_out(wbA)

        # to row-major
        alloc_rm = rmp.tile([RB, N], F32, tag="alloc_rm")
        for s in range(S_CHUNKS):
            nc.sync.dma_start(alloc_rm[:, s * FCH:(s + 1) * FCH], AL[s * 32:(s + 1) * 32, :])
        nc.sync.dma_start(g["alloc"].ap(), alloc_rm[:])

        # ------------------------------------------------------------------
        # cosine-sim stream (PE + ACT): memory tiles as stationary operand
        # ------------------------------------------------------------------
        dotT = rmp.tile([128, 16 * RB], F32, tag="dotT")
        mnT = rmp.tile([128, 16 * RB], F32, tag="mnT")
        _skip_sim = bool(int(os.environ.get("MWH_SKIP_SIM", "0")))
        with tc.tile_pool(name="simps", bufs=2, space="PSUM") as simps:
            for b in (() if _skip_sim else range(RB)):
                mem_sb = memp.tile([M, N], F32, tag="mem")
                nc.sync.dma_start(mem_sb[:], g["memT"].ap()[b])
                sq_sb = sqp.tile([M, N], F32, tag="sq")
                nc.scalar.activation(sq_sb[:], mem_sb[:], Act.Square)
                ps_d = simps.tile([128, 16], F32, tag="psd")
                ps_m = simps.tile([128, 16], F32, tag="psm")
                for c in range(16):
                    nc.tensor.matmul(ps_d[:, c:c + 1],
                                     mem_sb[:, c * 128:(c + 1) * 128],
                                     wkT_sb[:, b:b + 1], start=True, stop=True)
                    nc.tensor.matmul(ps_m[:, c:c + 1],
                                     sq_sb[:, c * 128:(c + 1) * 128],
                                     ones_col[:], start=True, stop=True)
                nc.scalar.copy(dotT[:, b * 16:(b + 1) * 16], ps_d[:])
                nc.scalar.copy(mnT[:, b * 16:(b + 1) * 16], ps_m[:])
        # transpose [128, (b c)] -> row-major [32, (c p)]
        dot_rm = rmp.tile([RB, N], F32, tag="dot_rm")
        mn_rm = rmp.tile([RB, N], F32, tag="mn_rm")
        if _skip_sim:
            nc.gpsimd.memset(dotT[:], 0.5)
            nc.gpsimd.memset(mnT[:], 0.5)
        with tc.tile_pool(name="simps2", bufs=1, space="PSUM") as simps2:
            for (src, dst, tg) in ((dotT, dot_rm, "d"), (mnT, mn_rm, "m")):
                ps_t = simps2.tile([RB, N], F32, tag=f"pst{tg}")
                for c in range(16):
                    nc.tensor.transpose(
                        ps_t[:, c * 128:(c + 1) * 128],
                        src[:].rearrange("p (b c) -> p b c", c=16)[:, :, c],
                        ident[:])
                nc.scalar.copy(dst[:], ps_t[:])

        # ------------------------------------------------------------------
        # epilogue: softmax + combine (row-major [32, 2048], tiles reused)
        # ------------------------------------------------------------------
        rm_s = rmp.tile([RB, N], F32, tag="rm_s")
        # mn_rm -> m_norm (in place), then denom into rm_s, recip into mn_rm
        nc.scalar.activation(mn_rm[:], mn_rm[:], Act.Sqrt)
        nc.vector.tensor_scalar(rm_s[:], mn_rm[:], k_norm[:], 1e-8, Alu.mult, Alu.add)
        nc.vector.reciprocal(mn_rm[:], rm_s[:])
        # z = dot * recip * strength (in place in dot_rm)
        nc.vector.scalar_tensor_tensor(dot_rm[:], dot_rm[:], strength[:], mn_rm[:],
                                       Alu.mult, Alu.mult)
        rowmax = rmp.tile([RB, 1], F32, tag="rowmax")
        nc.vector.tensor_reduce(out=rowmax[:], in_=dot_rm[:], axis=mybir.AxisListType.X, op=Alu.max)
        negmax = rmp.tile([RB, 1], F32, tag="negmax")
        nc.vector.tensor_scalar_mul(negmax[:], rowmax[:], -1.0)
        rowsum = rmp.tile([RB, 1], F32, tag="rowsum")
        nc.scalar.activation(rm_s[:], dot_rm[:], Act.Exp, bias=negmax[:], accum_out=rowsum[:])
        rs_rec = rmp.tile([RB, 1], F32, tag="rs_rec")
        nc.vector.reciprocal(rs_rec[:], rowsum[:])
        nc.vector.tensor_scalar(rs_rec[:], rs_rec[:], om_ag[:], None, Alu.mult)
        # content*(1-ag) into rm_s; alloc*ag into mn_rm; combine; *wg
        nc.vector.tensor_scalar(rm_s[:], rm_s[:], rs_rec[:], None, Alu.mult)
        nc.vector.scalar_tensor_tensor(rm_s[:], alloc_rm[:], agate[:], rm_s[:],
                                       Alu.mult, Alu.add)
        nc.vector.tensor_scalar(rm_s[:], rm_s[:], wgate[:], None, Alu.mult)
        nc.sync.dma_start(g["ww"].ap(), rm_s[:])


_PROGRAM = None


def _get_program():
    global _PROGRAM
    if _PROGRAM is None:
        nc = bacc.Bacc(None, target_bir_lowering=False, debug=False, num_devices=NCORES)
        with tile.TileContext(nc) as tc:
            _emit(nc, tc)
        nc.compile()
        _PROGRAM = nc
    return _PROGRAM


def _idx_const():
    p = np.arange(128)[:, None]
    f = np.arange(FCH)[None, :]
    return ((p >> 5) * FCH + f).astype(np.float32)


def _shard_inputs(inputs):
    idx = _idx_const()
    memT = np.ascontiguousarray(np.transpose(np.asarray(inputs["memory"]), (0, 2, 1)))
    in_maps = []
    for c in range(NCORES):
        sl = slice(c * RB, (c + 1) * RB)
        m = {
            "h": np.ascontiguousarray(np.asarray(inputs["h"])[sl]),
            "memT": np.ascontiguousarray(memT[sl]),
            "usage": np.ascontiguousarray(np.asarray(inputs["prev_usage"])[sl]),
            "idx": idx,
        }
        for k in ("Wk", "bk", "We", "be", "Wa", "ba", "Ws", "bs", "Wg", "bg", "Wag", "bag"):
            m[k] = np.ascontiguousarray(np.asarray(inputs[k], dtype=np.float32))
        in_maps.append(m)
    return in_maps


def kernel(**inputs):
    from concourse.bass_utils import run_bass_kernel_spmd

    nc = _get_program()
    in_maps = _shard_inputs(inputs)
    trace = bool(int(os.environ.get("MWH_TRACE", "0")))
    res = run_bass_kernel_spmd(nc, in_maps, list(range(NCORES)), trace=trace)
    if trace and res.exec_time_ns is not None:
        kernel.last_exec_time_ns = res.exec_time_ns
    ww = np.concatenate([res.results[c]["ww"] for c in range(NCORES)], axis=0)
    erase = np.concatenate([res.results[c]["erase"] for c in range(NCORES)], axis=0)
    addv = np.concatenate([res.results[c]["addv"] for c in range(NCORES)], axis=0)
    alloc = np.concatenate([res.results[c]["alloc"] for c in range(NCORES)], axis=0)
    return ww, erase, addv, alloc


# revision 26
# speedup vs baseline: 1.0607x; 1.0247x over previous
"""Trainium2 Bass kernel for nn_MemoryWriteHead (DNC-style memory write head).

Computes, per batch row:
  - tiny projections of h (write key/strength/erase/add/gates)
  - cosine content addressing over memory (B, N, M) + softmax
  - DNC allocation weighting: argsort usage, cumprod, scatter back
  - write_weights = wg * (ag * alloc + (1 - ag) * content)

Strategy: pure data parallel over 8 NeuronCores (32 batch rows each).
The argsort is an all-ascending bitonic sorting network on the DVE
(values sorted exactly on the 2^-23 uniform grid), with an index payload
moved by copy_predicated; per-stage swap masks are archived and replayed
in reverse to scatter allocation weights back to original positions.
Ties are fixed exactly with odd-even transposition passes on the index
payload. The cosine-sim dot products run on the TensorEngine against a
host-pre-transposed memory layout (m on partitions), so no on-chip
transposes of the big tensor are needed.
"""

import os
import sys

sys.path.insert(0, "/opt/trn_rl_repo")

import numpy as np

import concourse.bass as bass
import concourse.bacc as bacc
import concourse.mybir as mybir
import concourse.tile as tile
from concourse.masks import make_identity

B, N, M, H = 256, 2048, 128, 512
NCORES = 8
RB = B // NCORES          # 32 rows per core
S_CHUNKS = 4              # sort layout: 4 chunks of 512 per row
FCH = N // S_CHUNKS       # 512 free elems per chunk
TIE_PASSES = 1
V_SCALE = float(2.0 ** 23)
F32 = mybir.dt.float32
BF16 = mybir.dt.bfloat16
U8 = mybir.dt.uint8
Alu = mybir.AluOpType
Act = mybir.ActivationFunctionType


# ----------------------------------------------------------------------------
# sorting network description (python-side metadata)
# ----------------------------------------------------------------------------

def _sort_specs():
    """Instruction specs for the all-ascending bitonic network on 2048 elems.

    Element e = s*512 + f lives at partition s*32+r (r = row), free col f.
    Each spec: (kind, args) where kind selects the AP construction:
      ('xor_f', d)            pairs (f, f+d) within chunk, all partitions
      ('xor_p', pa, pb, cnt)  partition-offset pairs, full free dim
      ('refl_f', blk)         reflected pairs within free-dim blocks
      ('refl_p', pa, pb, cnt) partition pairs with reversed free dim
    Returns list of stages; each stage is a list of specs.
    """
    stages = []
    for k in range(1, 12):
        blk = 1 << k
        if blk == 2:
            stages.append([("xor_f", 1)])
        elif blk <= FCH:
            stages.append([("refl_f", blk)])
        elif blk == 2 * FCH:   # blocks (s0,s1), (s2,s3)
            stages.append([("refl_p", 0, 32, 32), ("refl_p", 64, 96, 32)])
        else:                  # blk == 4*FCH: pairs s0<->s3, s1<->s2
            stages.append([("refl_p", 0, 96, 32), ("refl_p", 32, 64, 32)])
        d = blk // 4
        while d >= 1:
            if d <= FCH // 2:
                stages.append([("xor_f", d)])
            else:  # d == 512: s-distance 1
                stages.append([("xor_p", 0, 32, 32), ("xor_p", 64, 96, 32)])
            d //= 2
    return stages


def _tie_specs():
    """Odd-even transposition passes over sorted slots (index payload only)."""
    stages = []
    for _ in range(TIE_PASSES):
        stages.append([("tie_even",)])
        stages.append([("tie_odd",), ("tie_bound",)])
    return stages


def _spec_aps(spec, t):
    """Given a spec and a [128, FCH] tile AP, return an op descriptor dict.

    Keys:
      A: AP for the low side (on the real tile)
      B: AP for the high side (real tile) -- only when stage is None
      mshape: mask tile shape
      mview: fn(mask_tile) -> AP matching A's dims
      stage: None, or dict(pa, cnt, src_off, w, rev) -- the B side lives at a
        different base partition; it must be DMA-staged into a scratch tile
        at base partition pa before ops, and DMA'd back after.
        src(t) gives the B-side slice on the real tile.
    """
    kind = spec[0]
    if kind == "xor_f":
        d = spec[1]
        v = t[:].rearrange("p (x two d) -> p x two d", two=2, d=d)
        x = FCH // (2 * d)
        mv = (lambda mk, d=d, x=x: mk[:].rearrange("p (d x) -> p x d", x=x))
        fz = None if d >= FCH // 2 else dict(
            ov=(lambda t, d=d: t[:].rearrange("p (x two d) -> p two x d", two=2, d=d)),
            pv=(lambda t, d=d: t[:].rearrange("p (x two d) -> p two x d",
                                              two=2, d=d)[:, ::-1, :, :]),
            mv0=(lambda mk, d=d, x=x: mk[:].rearrange("p (d x) -> p x d", x=x)
                 .unsqueeze(1).broadcast_to([128, 2, x, d])),
        )
        return dict(A=v[:, :, 0, :], B=v[:, :, 1, :], mshape=(128, FCH // 2),
                    mview=mv, stage=None, fused=fz)
    if kind == "refl_f":
        blk = spec[1]
        v = t[:].rearrange("p (x blk) -> p x blk", blk=blk)
        a = v[:, :, 0:blk // 2]
        b = v[:, :, blk - 1:blk // 2 - 1:-1]
        x = FCH // blk
        mv = (lambda mk, x=x: mk[:].rearrange("p (h x) -> p x h", x=x))
        return dict(A=a, B=b, mshape=(128, FCH // 2), mview=mv, stage=None)
    if kind == "tie_even":
        v = t[:].rearrange("p (x two) -> p x two", two=2)
        fz = dict(
            ov=(lambda t: t[:].rearrange("p (x two) -> p two x", two=2)),
            pv=(lambda t: t[:].rearrange("p (x two) -> p two x", two=2)[:, ::-1, :]),
            mv0=(lambda mk: mk[:].unsqueeze(1).broadcast_to([128, 2, FCH // 2])),
        )
        return dict(A=v[:, :, 0], B=v[:, :, 1], mshape=(128, FCH // 2),
                    mview=(lambda mk: mk[:]), stage=None, fused=fz)
    if kind == "tie_odd":
        v = t[:, 1:FCH - 1].rearrange("p (x two) -> p x two", two=2)
        x2 = (FCH - 2) // 2
        fz = dict(
            ov=(lambda t: t[:, 1:FCH - 1].rearrange("p (x two) -> p two x", two=2)),
            pv=(lambda t: t[:, 1:FCH - 1].rearrange("p (x two) -> p two x",
                                                    two=2)[:, ::-1, :]),
            mv0=(lambda mk, x2=x2: mk[:].unsqueeze(1).broadcast_to([128, 2, x2])),
        )
        return dict(A=v[:, :, 0], B=v[:, :, 1], mshape=(128, (FCH - 2) // 2),
                    mview=(lambda mk: mk[:]), stage=None, fused=fz)
    if kind == "xor_p":
        _, pa, pb, cnt = spec
        return dict(A=t[pa:pa + cnt, :], mshape=(128, FCH),
                    mview=(lambda mk, pa=pa, cnt=cnt: mk[pa:pa + cnt, :]),
                    stage=dict(pa=pa, cnt=cnt, src=t[pb:pb + cnt, :], w=FCH, rev=False))
    if kind == "refl_p":
        _, pa, pb, cnt = spec
        return dict(A=t[pa:pa + cnt, :], mshape=(128, FCH),
                    mview=(lambda mk, pa=pa, cnt=cnt: mk[pa:pa + cnt, :]),
                    stage=dict(pa=pa, cnt=cnt, src=t[pb:pb + cnt, :], w=FCH, rev=True))
    if kind == "tie_bound":
        return dict(A=t[0:96, FCH - 1:FCH], mshape=(128, 1),
                    mview=(lambda mk: mk[0:96, :]),
                    stage=dict(pa=0, cnt=96, src=t[32:128, 0:1], w=1, rev=False))
    raise ValueError(spec)


# ----------------------------------------------------------------------------
# program construction
# ----------------------------------------------------------------------------

def _emit(nc, tc):
    g = {}  # named DRAM tensors

    def din(name, shape):
        g[name] = nc.dram_tensor(name, shape, F32, kind="ExternalInput")

    def dout(name, shape):
        g[name] = nc.dram_tensor(name, shape, F32, kind="ExternalOutput")

    din("h", [RB, H])
    din("memT", [RB, M, N])
    din("usage", [RB, N])
    for w, b in (("Wk", "bk"), ("We", "be"), ("Wa", "ba")):
        din(w, [M, H])
        din(b, [M])
    for w, b in (("Ws", "bs"), ("Wg", "bg"), ("Wag", "bag")):
        din(w, [1, H])
        din(b, [1])
    din("idx", [128, FCH])
    dout("ww", [RB, N])
    dout("erase", [RB, M])
    dout("addv", [RB, M])
    dout("alloc", [RB, N])

    sort_stages = _sort_specs()
    tie_stages = _tie_specs()

    from contextlib import ExitStack
    es = ExitStack()
    with es:
        const = es.enter_context(tc.tile_pool(name="const", bufs=1))
        sortp = es.enter_context(tc.tile_pool(name="sortp", bufs=1))
        maskp = es.enter_context(tc.tile_pool(name="maskp", bufs=1))
        scr = es.enter_context(tc.tile_pool(name="scr", bufs=6))
        memp = es.enter_context(tc.tile_pool(name="memp", bufs=4))
        sqp = es.enter_context(tc.tile_pool(name="sqp", bufs=2))
        rmp = es.enter_context(tc.tile_pool(name="rmp", bufs=1))

        # ------------------------------------------------------------------
        # constants
        # ------------------------------------------------------------------
        ident = const.tile([128, 128], F32, tag="ident")
        make_identity(nc, ident[:])
        ones_col = const.tile([128, 1], F32, tag="ones")
        nc.gpsimd.memset(ones_col[:], 1.0)

        # ------------------------------------------------------------------
        # projections (PE)
        # ------------------------------------------------------------------
        with tc.tile_pool(name="prjps", bufs=2, space="PSUM") as prjps, \
             tc.tile_pool(name="prjacc", bufs=1, space="PSUM") as prjacc, \
             tc.tile_pool(name="prjsb", bufs=1) as prjsb:
            h_sb = prjsb.tile([RB, H], F32, tag="h")
            nc.sync.dma_start(h_sb[:], g["h"].ap())
            # hT: 4 transposes of [32,128] -> [128,32]
            hT = prjsb.tile([128, 4 * RB], F32, tag="hT")
            for kk in range(4):
                ps = prjps.tile([128, RB], F32, tag="tp", padded_shape=[128, 128])
                nc.tensor.transpose(ps[:], h_sb[:, kk * 128:(kk + 1) * 128], ident[0:RB, 0:RB])
                nc.scalar.copy(hT[:, kk * RB:(kk + 1) * RB], ps[:])

            # W transposes for Wk/We/Wa
            wT = {}
            for w in ("Wk", "We", "Wa"):
                w_sb = prjsb.tile([M, H], F32, tag="wld")
                nc.sync.dma_start(w_sb[:], g[w].ap())
                wt = prjsb.tile([128, 4 * 128], F32, tag=f"{w}T")
                for kk in range(4):
                    ps = prjps.tile([128, 128], F32, tag="tp")
                    nc.tensor.transpose(ps[:], w_sb[:, kk * 128:(kk + 1) * 128], ident[:])
                    nc.scalar.copy(wt[:, kk * 128:(kk + 1) * 128], ps[:])
                wT[w] = wt

            # scalar-projection weight columns: [128, 3] per k-tile
            svec = prjsb.tile([128, 4 * 3], F32, tag="svec")
            for j, w in enumerate(("Ws", "Wg", "Wag")):
                for kk in range(4):
                    nc.sync.dma_start(
                        svec[:, kk * 3 + j:kk * 3 + j + 1],
                        g[w].ap()[:, kk * 128:(kk + 1) * 128].rearrange("o k -> k o"),
                    )

            # biases
            bk128 = prjsb.tile([128, 1], F32, tag="bk128")
            nc.sync.dma_start(bk128[:], g["bk"].ap().rearrange("(m o) -> m o", o=1))
            brow = {}
            for b in ("bk", "be", "ba"):
                t = prjsb.tile([RB, M], F32, tag=f"{b}row")
                nc.sync.dma_start(t[:], g[b].ap().rearrange("(o m) -> o m", o=1).broadcast_to([RB, M]))
                brow[b] = t
            bsc = {}
            for b in ("bs", "bg", "bag"):
                t = prjsb.tile([RB, 1], F32, tag=f"{b}c")
                nc.sync.dma_start(t[:], g[b].ap().rearrange("(o p) -> o p", p=1).broadcast_to([RB, 1]))
                bsc[b] = t

            # matmuls
            ps_wkT = prjacc.tile([128, RB], F32, tag="wkT")
            ps_rm = {w: prjacc.tile([RB, 128], F32, tag=f"rm{w}", name=f"ps_rm_{w}") for w in ("Wk", "We", "Wa")}
            ps_sv = prjacc.tile([RB, 3], F32, tag="sv")
            for kk in range(4):
                hT_k = hT[:, kk * RB:(kk + 1) * RB]
                nc.tensor.matmul(ps_wkT[:], wT["Wk"][:, kk * 128:(kk + 1) * 128], hT_k,
                                 start=(kk == 0), stop=(kk == 3))
                for w in ("Wk", "We", "Wa"):
                    nc.tensor.matmul(ps_rm[w][:], hT_k, wT[w][:, kk * 128:(kk + 1) * 128],
                                     start=(kk == 0), stop=(kk == 3))
                nc.tensor.matmul(ps_sv[:], hT_k, svec[:, kk * 3:(kk + 1) * 3],
                                 start=(kk == 0), stop=(kk == 3))

            # epilogues of projections
            wkT_sb = const.tile([128, RB], F32, tag="wkT_sb")   # keep for sim phase
            nc.vector.tensor_scalar(wkT_sb[:], ps_wkT[:], bk128[:], None, Alu.add)

            wk_rm = rmp.tile([RB, 128], F32, tag="wk_rm")
            nc.vector.tensor_tensor(wk_rm[:], ps_rm["Wk"][:], brow["bk"][:], Alu.add)
            kn_scr = rmp.tile([RB, 128], F32, tag="kn_scr")
            k_nsq = rmp.tile([RB, 1], F32, tag="k_nsq")
            nc.scalar.activation(kn_scr[:], wk_rm[:], Act.Square, accum_out=k_nsq[:])
            k_norm = rmp.tile([RB, 1], F32, tag="k_norm")
            nc.scalar.activation(k_norm[:], k_nsq[:], Act.Sqrt)

            er_t = rmp.tile([RB, 128], F32, tag="er_t")
            nc.vector.tensor_tensor(er_t[:], ps_rm["We"][:], brow["be"][:], Alu.add)
            er_o = rmp.tile([RB, 128], F32, tag="er_o")
            nc.scalar.activation(er_o[:], er_t[:], Act.Sigmoid)
            nc.sync.dma_start(g["erase"].ap(), er_o[:])

            ad_o = rmp.tile([RB, 128], F32, tag="ad_o")
            nc.vector.tensor_tensor(ad_o[:], ps_rm["Wa"][:], brow["ba"][:], Alu.add)
            nc.sync.dma_start(g["addv"].ap(), ad_o[:])

            strength = rmp.tile([RB, 1], F32, tag="strength")
            nc.scalar.activation(strength[:], ps_sv[:, 0:1], Act.Exp, bias=bsc["bs"][:])
            nc.vector.tensor_scalar_add(strength[:], strength[:], 1.0)
            nc.scalar.activation(strength[:], strength[:], Act.Ln)
            wgate = rmp.tile([RB, 1], F32, tag="wgate")
            nc.scalar.activation(wgate[:], ps_sv[:, 1:2], Act.Sigmoid, bias=bsc["bg"][:])
            agate = rmp.tile([RB, 1], F32, tag="agate")
            nc.scalar.activation(agate[:], ps_sv[:, 2:3], Act.Sigmoid, bias=bsc["bag"][:])
            om_ag = rmp.tile([RB, 1], F32, tag="om_ag")
            nc.vector.tensor_scalar(om_ag[:], agate[:], -1.0, 1.0, Alu.mult, Alu.add)

        # ------------------------------------------------------------------
        # sort phase (DVE + ACT)
        # ------------------------------------------------------------------
        V = sortp.tile([128, FCH], F32, tag="V")
        nc.sync.dma_start(V[:], g["usage"].ap().rearrange("r (s f) -> s r f", s=S_CHUNKS))
        nc.vector.tensor_scalar_mul(V[:], V[:], V_SCALE)
        I = sortp.tile([128, FCH], F32, tag="I")
        nc.sync.dma_start(I[:], g["idx"].ap())

        all_replay = []  # (spec, mask_tile)

        _scr_n = [0]

        def scratch(tag, shape=(128, FCH)):
            _scr_n[0] += 1
            return scr.tile(list(shape), F32, tag=tag, name=f"scr{_scr_n[0]}",
                            padded_shape=[128, FCH])

        def stage_in(sp, tile_ap):
            """DMA the B side into a base-aligned scratch; return (ops_B_ap, writeback)."""
            st = sp["stage"]
            if st is None:
                return sp["B"], None
            stg = scratch("stg")
            pa, cnt, w = st["pa"], st["cnt"], st["w"]
            dst = stg[pa:pa + cnt, 0:w]
            nc.sync.dma_start(dst, st["src"])
            if st["rev"]:
                ops_b = stg[pa:pa + cnt, w - 1::-1]
            else:
                ops_b = dst
            return ops_b, (dst, st["src"])

        def stage_out(wb):
            if wb is not None:
                dst, src = wb
                nc.sync.dma_start(src, dst)

        def emit_swap_stage(spec, is_tie):
            spV = _spec_aps(spec, V)
            spI = _spec_aps(spec, I)
            mshape, mview = spV["mshape"], spV["mview"]
            VA = spV["A"]
            IA = spI["A"]
            VB, wbV = stage_in(spV, V)
            IB, wbI = stage_in(spI, I)
            tag = "t" if is_tie else "m"
            mk = maskp.tile(list(mshape), U8, tag=f"{tag}{len(all_replay)}",
                            name=f"mk{len(all_replay)}")
            mkv = mview(mk)
            fz = spV.get("fused")
            if is_tie:
                eq = scratch("se", mshape)
                eqv = mview(eq)
                nc.vector.tensor_tensor(eqv, VA, VB, Alu.is_equal)
                gt = scratch("sg", mshape)
                gtv = mview(gt)
                nc.vector.tensor_tensor(gtv, IA, IB, Alu.is_gt)
                nc.vector.tensor_tensor(mkv, eqv, gtv, Alu.logical_and)
            else:
                nc.vector.tensor_tensor(mkv, VA, VB, Alu.is_gt)
                if fz is not None:
                    tv = scratch("sv")
                    nc.scalar.copy(tv[:], V[:])
                    nc.vector.copy_predicated(fz["ov"](V), fz["mv0"](mk), fz["pv"](tv))
                else:
                    tv = scratch("sv", mshape)
                    tvv = mview(tv)
                    nc.scalar.copy(tvv, VA)
                    nc.vector.tensor_tensor(VA, VA, VB, Alu.min)
                    nc.vector.tensor_tensor(VB, tvv, VB, Alu.max)
            if fz is not None:
                ti = scratch("si")
                nc.scalar.copy(ti[:], I[:])
                nc.vector.copy_predicated(fz["ov"](I), fz["mv0"](mk), fz["pv"](ti))
            else:
                ti = scratch("si", mshape)
                tiv = mview(ti)
                nc.scalar.copy(tiv, IA)
                nc.vector.copy_predicated(IA, mkv, IB)
                nc.vector.copy_predicated(IB, mkv, tiv)
            stage_out(wbV)
            stage_out(wbI)
            all_replay.append((spec, mk))

        for stage in sort_stages:
            for spec in stage:
                emit_swap_stage(spec, is_tie=False)
        for stage in tie_stages:
            for spec in stage:
                emit_swap_stage(spec, is_tie=True)

        # ------------------------------------------------------------------
        # cumprod + allocation weights in sorted space
        # ------------------------------------------------------------------
        s_val = sortp.tile([128, FCH], F32, tag="s_val")
        nc.vector.tensor_scalar_mul(s_val[:], V[:], 1.0 / V_SCALE)
        cp = sortp.tile([128, FCH], F32, tag="cp")
        nc.vector.tensor_tensor_scan(cp[:], s_val[:], s_val[:], 1.0, Alu.mult, Alu.bypass)
        # cross-chunk exclusive products, computed in a [32, 4] layout
        # (per-row chunk totals brought to one partition set via tiny DMAs)
        T32 = sortp.tile([32, 4], F32, tag="T32")
        for sc in range(S_CHUNKS):
            nc.sync.dma_start(T32[:, sc:sc + 1], cp[sc * 32:(sc + 1) * 32, FCH - 1:FCH])
        excT = sortp.tile([32, 4], F32, tag="excT")
        nc.gpsimd.memset(excT[:, 0:1], 1.0)
        nc.vector.tensor_copy(excT[:, 1:2], T32[:, 0:1])
        nc.vector.tensor_tensor(excT[:, 2:3], T32[:, 1:2], excT[:, 1:2], Alu.mult)
        nc.vector.tensor_tensor(excT[:, 3:4], T32[:, 2:3], excT[:, 2:3], Alu.mult)
        exc = sortp.tile([128, 1], F32, tag="exc")
        for sc in range(S_CHUNKS):
            nc.sync.dma_start(exc[sc * 32:(sc + 1) * 32, 0:1], excT[:, sc:sc + 1])
        # padded (exclusive within chunk)
        padded = sortp.tile([128, FCH], F32, tag="padded")
        nc.gpsimd.memset(padded[:, 0:1], 1.0)
        nc.vector.tensor_copy(padded[:, 1:FCH], cp[:, 0:FCH - 1])
        # alloc_sorted = (1 - s) * padded * exc
        AL = sortp.tile([128, FCH], F32, tag="AL")
        nc.vector.tensor_scalar(AL[:], padded[:], exc[:], None, Alu.mult)
        om_s = sortp.tile([128, FCH], F32, tag="om_s")
        nc.vector.tensor_scalar(om_s[:], s_val[:], -1.0, 1.0, Alu.mult, Alu.add)
        nc.vector.tensor_tensor(AL[:], om_s[:], AL[:], Alu.mult)

        # ------------------------------------------------------------------
        # replay (unsort alloc back to original positions)
        # ------------------------------------------------------------------
        _skip_replay = bool(int(os.environ.get("MWH_SKIP_REPLAY", "0")))
        for spec, mk in (() if _skip_replay else tuple(reversed(all_replay))):
            sp = _spec_aps(spec, AL)
            AA = sp["A"]
            mkv = sp["mview"](mk)
            AB, wbA = stage_in(sp, AL)
            ta = scratch("sa", sp["mshape"])
            tav = sp["mview"](ta)
            nc.vector.tensor_copy(tav, AA)
            nc.vector.copy_predicated(AA, mkv, AB)
            nc.vector.copy_predicated(AB, mkv, tav)
            stage_out(wbA)

        # to row-major
        alloc_rm = rmp.tile([RB, N], F32, tag="alloc_rm")
        for s in range(S_CHUNKS):
            nc.sync.dma_start(alloc_rm[:, s * FCH:(s + 1) * FCH], AL[s * 32:(s + 1) * 32, :])
        nc.sync.dma_start(g["alloc"].ap(), alloc_rm[:])

        # ------------------------------------------------------------------
        # cosine-sim stream (PE + ACT): memory tiles as stationary operand
        # ------------------------------------------------------------------
        dotT = rmp.tile([128, 16 * RB], F32, tag="dotT")
        mnT = rmp.tile([128, 16 * RB], F32, tag="mnT")
        _skip_sim = bool(int(os.environ.get("MWH_SKIP_SIM", "0")))
        with tc.tile_pool(name="simps", bufs=2, space="PSUM") as simps:
            for b in (() if _skip_sim else range(RB)):
                mem_sb = memp.tile([M, N], F32, tag="mem")
                nc.sync.dma_start(mem_sb[:], g["memT"].ap()[b])
                sq_sb = sqp.tile([M, N], F32, tag="sq")
                nc.gpsimd.tensor_tensor(sq_sb[:], mem_sb[:], mem_sb[:], Alu.mult)
                ps_d = simps.tile([128, 16], F32, tag="psd")
                ps_m = simps.tile([128, 16], F32, tag="psm")
                for c in range(16):
                    nc.tensor.matmul(ps_d[:, c:c + 1],
                                     mem_sb[:, c * 128:(c + 1) * 128],
                                     wkT_sb[:, b:b + 1], start=True, stop=True)
                    nc.tensor.matmul(ps_m[:, c:c + 1],
                                     sq_sb[:, c * 128:(c + 1) * 128],
                                     ones_col[:], start=True, stop=True)
                nc.scalar.copy(dotT[:, b * 16:(b + 1) * 16], ps_d[:])
                nc.scalar.copy(mnT[:, b * 16:(b + 1) * 16], ps_m[:])
        # transpose [128, (b c)] -> row-major [32, (c p)]
        dot_rm = rmp.tile([RB, N], F32, tag="dot_rm")
        mn_rm = rmp.tile([RB, N], F32, tag="mn_rm")
        if _skip_sim:
            nc.gpsimd.memset(dotT[:], 0.5)
            nc.gpsimd.memset(mnT[:], 0.5)
        with tc.tile_pool(name="simps2", bufs=1, space="PSUM") as simps2:
            for (src, dst, tg) in ((dotT, dot_rm, "d"), (mnT, mn_rm, "m")):
                ps_t = simps2.tile([RB, N], F32, tag=f"pst{tg}")
                for c in range(16):
                    nc.tensor.transpose(
                        ps_t[:, c * 128:(c + 1) * 128],
                        src[:].rearrange("p (b c) -> p b c", c=16)[:, :, c],
                        ident[:])
                nc.scalar.copy(dst[:], ps_t[:])

        # ------------------------------------------------------------------
        # epilogue: softmax + combine (row-major [32, 2048], tiles reused)
        # ------------------------------------------------------------------
        rm_s = rmp.tile([RB, N], F32, tag="rm_s")
        # mn_rm -> m_norm (in place), then denom into rm_s, recip into mn_rm
        nc.scalar.activation(mn_rm[:], mn_rm[:], Act.Sqrt)
        nc.vector.tensor_scalar(rm_s[:], mn_rm[:], k_norm[:], 1e-8, Alu.mult, Alu.add)
        nc.vector.reciprocal(mn_rm[:], rm_s[:])
        # z = dot * recip * strength (in place in dot_rm)
        nc.vector.scalar_tensor_tensor(dot_rm[:], dot_rm[:], strength[:], mn_rm[:],
                                       Alu.mult, Alu.mult)
        rowmax = rmp.tile([RB, 1], F32, tag="rowmax")
        nc.vector.tensor_reduce(out=rowmax[:], in_=dot_rm[:], axis=mybir.AxisListType.X, op=Alu.max)
        negmax = rmp.tile([RB, 1], F32, tag="negmax")
        nc.vector.tensor_scalar_mul(negmax[:], rowmax[:], -1.0)
        rowsum = rmp.tile([RB, 1], F32, tag="rowsum")
        nc.scalar.activation(rm_s[:], dot_rm[:], Act.Exp, bias=negmax[:], accum_out=rowsum[:])
        rs_rec = rmp.tile([RB, 1], F32, tag="rs_rec")
        nc.vector.reciprocal(rs_rec[:], rowsum[:])
        nc.vector.tensor_scalar(rs_rec[:], rs_rec[:], om_ag[:], None, Alu.mult)
        # content*(1-ag) into rm_s; alloc*ag into mn_rm; combine; *wg
        nc.vector.tensor_scalar(rm_s[:], rm_s[:], rs_rec[:], None, Alu.mult)
        nc.vector.scalar_tensor_tensor(rm_s[:], alloc_rm[:], agate[:], rm_s[:],
                                       Alu.mult, Alu.add)
        nc.vector.tensor_scalar(rm_s[:], rm_s[:], wgate[:], None, Alu.mult)
        nc.sync.dma_start(g["ww"].ap(), rm_s[:])


_PROGRAM = None


def _get_program():
    global _PROGRAM
    if _PROGRAM is None:
        nc = bacc.Bacc(None, target_bir_lowering=False, debug=False, num_devices=NCORES)
        with tile.TileContext(nc) as tc:
            _emit(nc, tc)
        nc.compile()
        _PROGRAM = nc
    return _PROGRAM


def _idx_const():
    p = np.arange(128)[:, None]
    f = np.arange(FCH)[None, :]
    return ((p >> 5) * FCH + f).astype(np.float32)


def _shard_inputs(inputs):
    idx = _idx_const()
    memT = np.ascontiguousarray(np.transpose(np.asarray(inputs["memory"]), (0, 2, 1)))
    in_maps = []
    for c in range(NCORES):
        sl = slice(c * RB, (c + 1) * RB)
        m = {
            "h": np.ascontiguousarray(np.asarray(inputs["h"])[sl]),
            "memT": np.ascontiguousarray(memT[sl]),
            "usage": np.ascontiguousarray(np.asarray(inputs["prev_usage"])[sl]),
            "idx": idx,
        }
        for k in ("Wk", "bk", "We", "be", "Wa", "ba", "Ws", "bs", "Wg", "bg", "Wag", "bag"):
            m[k] = np.ascontiguousarray(np.asarray(inputs[k], dtype=np.float32))
        in_maps.append(m)
    return in_maps


def kernel(**inputs):
    from concourse.bass_utils import run_bass_kernel_spmd

    nc = _get_program()
    in_maps = _shard_inputs(inputs)
    trace = bool(int(os.environ.get("MWH_TRACE", "0")))
    res = run_bass_kernel_spmd(nc, in_maps, list(range(NCORES)), trace=trace)
    if trace and res.exec_time_ns is not None:
        kernel.last_exec_time_ns = res.exec_time_ns
    ww = np.concatenate([res.results[c]["ww"] for c in range(NCORES)], axis=0)
    erase = np.concatenate([res.results[c]["erase"] for c in range(NCORES)], axis=0)
    addv = np.concatenate([res.results[c]["addv"] for c in range(NCORES)], axis=0)
    alloc = np.concatenate([res.results[c]["alloc"] for c in range(NCORES)], axis=0)
    return ww, erase, addv, alloc
